# revision 5
# baseline (speedup 1.0000x reference)
"""GaussianEmbedding Trainium2 Bass kernel (8-core data parallel).

out[b,t,:] = sum_s w[b,t,s] * embed[text[b,s]],  w = normalized Gaussian
weights centered at token centers c_s with sigma = dur_s/2.

Strategy:
  - Pure data parallel: 4 batch rows per core, embed table replicated.
  - Per (row, s-tile of 128 tokens) only the time band |t - c| <= R*sigma
    matters (Gaussian tails underflow); bands are computed on host from the
    actual durations (union across the 8 cores, since SPMD shares one
    program) and baked into the instruction stream.  A new input signature
    recompiles (cached by band signature).
  - On-device per (row, s-tile): z = (t - c')/sig via DVE tensor_scalar from
    an iota row, z^2 via GPSIMD tensor_tensor (or fused on ACT Square),
    w = exp(-0.5 z^2 - log sig - log sqrt(2pi)) via ACT Exp with
    per-partition bias.  w lives [s partitions, t free] = matmul lhsT.
  - Embedding gather on device via one-hot matmul: onehot[i,s] built with a
    PE broadcast of the token ids + DVE is_equal; emb_g = onehot.T @ embed
    (embed shipped with an appended ones column -> emb_g[s,256] = 1, so the
    main matmul's PSUM column 256 is the normalization denominator).
  - Main matmul per output t-tile accumulates only contributing s-tiles.
    Normalization: recip(denom + eps) then PSUM->SBUF copy scaled by the
    per-frame reciprocal (split across DVE and ACT), bf16 out, one DMA per
    4 t-tiles.
  - Frames t >= total duration are zeroed on host (reference semantics).
"""

import numpy as np

# Problem constants (kernel.py is self-contained; shapes hardcoded).
B, S, IDIM, D, T = 32, 512, 256, 256, 4096
EPS = 1e-6
SIGMA_C = 2.0
PAD = 0
LOG_SQRT_2PI = 0.9189385332046727
N_CORES = 8
NR = B // N_CORES          # rows per core
NK = S // 128              # s-tiles per row
R_BAND = 6.0               # Gaussian cutoff in sigmas

# Engine assignment (tunable): per (r,k) w-tile mode and normcopy split.
# 'A': ACT Square; 'D': DVE z + GPSIMD square; 'G': GPSIMD z + DVE square.
W_MODE = "DDAD" * 4
# normcopy engine cycle: 'V' = DVE tensor_scalar, 'A' = ACT copy-with-scale
NC_PAT = "VAVA"

_runners = {}


def _host_prep(text, durs):
    """Per-token params + union bands. Returns (params[8][128,64] fp32,
    textf[8][4,512] fp32, bands {(r,k):(lo,hi)}, ncov[r], L[B])."""
    durs = np.asarray(durs).astype(np.int64)
    text = np.asarray(text).astype(np.int64)
    excl = np.cumsum(durs, axis=-1) - durs
    L = np.minimum(np.cumsum(durs, axis=-1)[:, -1], T).astype(np.int64)
    durs_f = durs.astype(np.float64)
    c = durs_f / 2.0 + excl.astype(np.float64)
    sig = durs_f / SIGMA_C + EPS
    valid = (durs > 0) & (text != PAD)

    inv_sig = np.where(valid, 1.0 / sig, 1e6)
    cprime = np.where(valid, c - 0.5, -4e6)
    b2 = np.where(valid, -np.log(sig) - LOG_SQRT_2PI, 0.0)

    bands = {}
    for r in range(NR):
        for k in range(NK):
            lo, hi = float(T), 0.0
            for cid in range(N_CORES):
                b = cid * NR + r
                sl = slice(k * 128, (k + 1) * 128)
                m = valid[b, sl]
                if not m.any():
                    continue
                cs, ss = c[b, sl][m], sig[b, sl][m]
                lo = min(lo, (cs - R_BAND * ss).min())
                hi = max(hi, (cs + R_BAND * ss).max())
            if hi <= lo:
                bands[(r, k)] = None
                continue
            lo = int(max(0, np.floor(lo / 128.0) * 128))
            hi = int(min(T, np.ceil(hi / 128.0) * 128))
            bands[(r, k)] = (lo, hi)
    ncov = []
    for r in range(NR):
        hi_r = max((bands[(r, k)][1] for k in range(NK) if bands[(r, k)]), default=0)
        ncov.append(hi_r // 128)

    # Safety: every frame t < L_b must be covered by some band of its row.
    for b in range(B):
        r = b % NR
        need = min(int(np.ceil(L[b] / 128.0)), T // 128)
        for tt in range(need):
            ok = any(
                bands[(r, k)] and bands[(r, k)][0] < (tt + 1) * 128
                and bands[(r, k)][1] > tt * 128
                for k in range(NK)
            )
            if not ok:  # pragma: no cover - fall back to dense bands
                for k in range(NK):
                    bands[(r, k)] = (0, T)
                ncov[r] = T // 128
                break

    params = np.zeros((N_CORES, 128, NR * NK * 4), dtype=np.float32)
    textf = np.zeros((N_CORES, NR, S), dtype=np.float32)
    for cid in range(N_CORES):
        for r in range(NR):
            b = cid * NR + r
            textf[cid, r] = text[b].astype(np.float32)
            for k in range(NK):
                sl = slice(k * 128, (k + 1) * 128)
                j = (r * NK + k) * 4
                params[cid, :, j + 0] = cprime[b, sl]
                params[cid, :, j + 1] = inv_sig[b, sl]
                params[cid, :, j + 2] = -cprime[b, sl] * inv_sig[b, sl]
                params[cid, :, j + 3] = b2[b, sl]
    return params, textf, bands, ncov, L


def _build_program(bands, ncov, reps=1):
    import concourse.bacc as bacc
    import concourse.mybir as mybir
    import concourse.tile as tile

    dt = mybir.dt
    Act = mybir.ActivationFunctionType
    Alu = mybir.AluOpType

    nc = bacc.Bacc("TRN2", target_bir_lowering=False, debug=False,
                   num_devices=N_CORES)
    d_params = nc.dram_tensor("params", [128, NR * NK * 4], dt.float32,
                              kind="ExternalInput").ap()
    d_idcol = nc.dram_tensor("idcol", [128, 1], dt.float32,
                             kind="ExternalInput").ap()
    d_textf = nc.dram_tensor("textf", [NR, S], dt.float32,
                             kind="ExternalInput").ap()
    d_embaug = nc.dram_tensor("embaug", [IDIM, D + 1], dt.bfloat16,
                              kind="ExternalInput").ap()
    d_out = nc.dram_tensor("out", [NR, T, D], dt.bfloat16,
                           kind="ExternalOutput").ap()

    with tile.TileContext(nc) as tc:
        with tc.tile_pool(name="const", bufs=1) as cpool:
            iota_t = cpool.tile([128, T], dt.float32, tag="iota")
            nc.gpsimd.iota(iota_t[:, :], pattern=[[1, T]], base=0,
                           channel_multiplier=0,
                           allow_small_or_imprecise_dtypes=True)
            params_t = cpool.tile([128, NR * NK * 4], dt.float32, tag="par")
            nc.sync.dma_start(out=params_t[:, :], in_=d_params[:, :])
            idcol_t = cpool.tile([128, 1], dt.float32, tag="idc")
            nc.sync.dma_start(out=idcol_t[:, :], in_=d_idcol[:, :])
            embaug_t = cpool.tile([128, 2, D + 1], dt.bfloat16, tag="emb")
            nc.sync.dma_start(
                out=embaug_t[:, :, :],
                in_=d_embaug.rearrange("(c p) n -> p c n", p=128))
            ones1 = cpool.tile([1, 128], dt.float32, tag="ones")
            nc.gpsimd.memset(ones1[:, :], 1.0)

            for rep in range(reps):
                d_o = d_out if rep == 0 else nc.dram_tensor(
                    f"scratch_out{rep}", [NR, T, D], dt.bfloat16).ap()
                _emit_body(nc, tc, tile, dt, Act, Alu, bands, ncov,
                           d_textf, d_o, iota_t, params_t, idcol_t,
                           embaug_t, ones1, rep)
    nc.compile()
    return nc


def _emit_body(nc, tc, tile, dt, Act, Alu, bands, ncov, d_textf, d_out,
               iota_t, params_t, idcol_t, embaug_t, ones1, rep):
    from contextlib import ExitStack

    with ExitStack() as ctx:
        embg_pool = ctx.enter_context(
            tc.tile_pool(name=f"embg{rep}", bufs=NR * NK))
        sbuf = ctx.enter_context(tc.tile_pool(name=f"work{rep}", bufs=3))
        wpool = ctx.enter_context(tc.tile_pool(name=f"wt{rep}", bufs=2 * NK))

        # ---- embedding gather via one-hot matmuls ----
        embg = {}
        with tc.tile_pool(name=f"psg{rep}", bufs=2, space="PSUM") as psg:
            for r in range(NR):
                trow = sbuf.tile([1, S], dt.float32, tag="trow")
                nc.sync.dma_start(out=trow[:, :], in_=d_textf[r:r + 1, :])
                tb = psg.tile([128, S], dt.float32, tag="tb")
                nc.tensor.matmul(tb[:, :], ones1[:, :], trow[:, :],
                                 start=True, stop=True)
                oh = []
                for cch in range(IDIM // 128):
                    o = sbuf.tile([128, S], dt.bfloat16, tag="oh")
                    nc.vector.tensor_scalar(o[:, :], tb[:, :],
                                            idcol_t[:, 0:1], float(128 * cch),
                                            Alu.subtract, Alu.is_equal)
                    oh.append(o)
                for k in range(NK):
                    eg = psg.tile([128, D + 1], dt.float32, tag="eg")
                    for cch in range(IDIM // 128):
                        nc.tensor.matmul(
                            eg[:, :], oh[cch][:, k * 128:(k + 1) * 128],
                            embaug_t[:, cch, :],
                            start=(cch == 0), stop=(cch == 1))
                    g = embg_pool.tile([128, D + 1], dt.bfloat16, tag="embg")
                    nc.vector.tensor_copy(g[:, :], eg[:, :])
                    embg[(r, k)] = g

        # ---- w tiles ----
        wt = {}
        for r in range(NR):
            for k in range(NK):
                if bands[(r, k)] is None:
                    continue
                lo, hi = bands[(r, k)]
                W = hi - lo
                j = (r * NK + k) * 4
                mode = W_MODE[r * NK + k]
                sq = sbuf.tile([128, W], dt.bfloat16, tag="sq")
                if mode == "A":
                    nc.scalar.activation(sq[:, :], iota_t[:, lo:hi],
                                         Act.Square,
                                         bias=params_t[:, j + 2:j + 3],
                                         scale=params_t[:, j + 1:j + 2])
                else:
                    z = sbuf.tile([128, W], dt.bfloat16, tag="z")
                    zeng = nc.vector if mode == "D" else nc.gpsimd
                    seng = nc.gpsimd if mode == "D" else nc.vector
                    zeng.tensor_scalar(z[:, :], iota_t[:, lo:hi],
                                       params_t[:, j + 0:j + 1],
                                       params_t[:, j + 1:j + 2],
                                       Alu.subtract, Alu.mult)
                    seng.tensor_tensor(sq[:, :], z[:, :], z[:, :], Alu.mult)
                w = wpool.tile([128, W], dt.bfloat16, tag="w")
                nc.scalar.activation(w[:, :], sq[:, :], Act.Exp,
                                     bias=params_t[:, j + 3:j + 4],
                                     scale=-0.5)
                wt[(r, k)] = (w, lo, hi)

        # ---- main matmuls + normalize + out DMA ----
        with tc.tile_pool(name=f"psm{rep}", bufs=2, space="PSUM") as psm:
            nci = 0
            for r in range(NR):
                for g0 in range(0, ncov[r], 4):
                    g = min(4, ncov[r] - g0)
                    big = psm.tile([128, 4 * 512], dt.float32, tag="big")
                    for q in range(g):
                        tt = g0 + q
                        ks = [k for k in range(NK)
                              if bands[(r, k)]
                              and bands[(r, k)][0] < (tt + 1) * 128
                              and bands[(r, k)][1] > tt * 128]
                        dst = big[:, q * 512:q * 512 + D + 1]
                        for ji, k in enumerate(ks):
                            w, lo, hi = wt[(r, k)]
                            off = tt * 128 - lo
                            nc.tensor.matmul(dst, w[:, off:off + 128],
                                             embg[(r, k)][:, :],
                                             start=(ji == 0),
                                             stop=(ji == len(ks) - 1))
                    den = sbuf.tile([128, g], dt.float32, tag="den")
                    nc.vector.tensor_scalar(
                        den[:, :], big[:, D:D + (g - 1) * 512 + 1:512],
                        EPS, None, Alu.add)
                    rec = sbuf.tile([128, g], dt.float32, tag="rec")
                    nc.vector.reciprocal(rec[:, :], den[:, :])
                    stage = sbuf.tile([128, 4 * D], dt.bfloat16, tag="stage")
                    for q in range(g):
                        e = NC_PAT[nci % len(NC_PAT)]
                        nci += 1
                        src = big[:, q * 512:q * 512 + D]
                        dst = stage[:, q * D:(q + 1) * D]
                        if e == "V":
                            nc.vector.tensor_scalar(dst, src, rec[:, q:q + 1],
                                                    None, Alu.mult)
                        else:
                            nc.scalar.mul(dst, src, rec[:, q:q + 1])
                    nc.sync.dma_start(
                        out=d_out[r, g0 * 128:(g0 + g) * 128, :].rearrange(
                            "(q p) d -> p q d", p=128),
                        in_=stage[:, 0:g * D])


def _make_runner(nc):
    import jax
    import concourse.mybir as mybir
    from concourse import bass2jax
    from jax.experimental.shard_map import shard_map
    from jax.sharding import Mesh, PartitionSpec

    bass2jax.install_neuronx_cc_hook()

    partition_name = (nc.partition_id_tensor.name
                      if nc.partition_id_tensor else None)
    in_names, out_names, out_avals, out_shapes = [], [], [], []
    for alloc in nc.m.functions[0].allocations:
        if not isinstance(alloc, mybir.MemoryLocationSet):
            continue
        name = alloc.memorylocations[0].name
        if alloc.kind == "ExternalInput":
            if name != partition_name:
                in_names.append(name)
        elif alloc.kind == "ExternalOutput":
            out_names.append(name)
            shape = tuple(alloc.tensor_shape)
            dtype = mybir.dt.np(alloc.dtype)
            out_avals.append(jax.core.ShapedArray(shape, dtype))
            out_shapes.append((shape, dtype))
    n_params = len(in_names)
    all_in_names = list(in_names) + list(out_names)
    if partition_name is not None:
        all_in_names.append(partition_name)
    all_in_names = tuple(all_in_names)
    donate = tuple(range(n_params, n_params + len(out_names)))

    def _body(*args):
        operands = list(args)
        if partition_name is not None:
            operands.append(bass2jax.partition_id_tensor())
        outs = bass2jax._bass_exec_p.bind(
            *operands,
            out_avals=tuple(out_avals),
            in_names=all_in_names,
            out_names=tuple(out_names),
            lowering_input_output_aliases=(),
            sim_require_finite=True,
            sim_require_nnan=True,
            nc=nc,
        )
        return tuple(outs)

    devices = jax.devices()[:N_CORES]
    mesh = Mesh(np.asarray(devices), ("core",))
    specs = (PartitionSpec("core"),) * (n_params + len(out_names))
    out_specs = (PartitionSpec("core"),) * len(out_names)
    fn = jax.jit(
        shard_map(_body, mesh=mesh, in_specs=specs, out_specs=out_specs,
                  check_rep=False),
        donate_argnums=donate, keep_unused=True)

    def run(in_maps):
        concat_in = [
            np.concatenate([np.asarray(in_maps[c][n]) for c in range(N_CORES)],
                           axis=0)
            for n in in_names
        ]
        concat_zero = [
            np.zeros((N_CORES * sh[0], *sh[1:]), dtp)
            for (sh, dtp) in out_shapes
        ]
        out_arrs = fn(*concat_in, *concat_zero)
        return {
            name: np.asarray(out_arrs[i]).reshape(N_CORES, *out_shapes[i][0])
            for i, name in enumerate(out_names)
        }

    return run


def get_runner(key, bands, ncov, reps=1):
    k = (key, reps)
    if k not in _runners:
        nc = _build_program(bands, ncov, reps=reps)
        _runners[k] = _make_runner(nc)
    return _runners[k]


def kernel(text, durs, embed, total_time):
    import ml_dtypes

    text = np.asarray(text)
    durs = np.asarray(durs)
    embed = np.asarray(embed, dtype=np.float32)
    tt_ = int(np.asarray(total_time))
    assert tt_ == T and text.shape == (B, S) and embed.shape == (IDIM, D)

    params, textf, bands, ncov, L = _host_prep(text, durs)
    key = tuple(sorted((rk, v) for rk, v in bands.items())) + tuple(ncov)
    run = get_runner(key, bands, ncov)

    embaug = np.ones((IDIM, D + 1), dtype=ml_dtypes.bfloat16)
    embaug[:, :D] = embed.astype(ml_dtypes.bfloat16)
    embaug[PAD, :D] = 0
    idcol = np.arange(128, dtype=np.float32).reshape(128, 1)

    in_maps = [
        {"params": params[cid], "idcol": idcol, "textf": textf[cid],
         "embaug": embaug}
        for cid in range(N_CORES)
    ]
    res = run(in_maps)
    out = res["out"].astype(np.float32).reshape(B, T, D)
    for b in range(B):
        out[b, L[b]:, :] = 0.0
    return out


# revision 17
# speedup vs baseline: 864.1395x; 864.1395x over previous
"""GaussianEmbedding Trainium2 Bass kernel (8-core data parallel).

out[b,t,:] = sum_s w[b,t,s] * embed[text[b,s]],  w = normalized Gaussian
weights centered at token centers c_s with sigma = dur_s/2.

Strategy:
  - Pure data parallel: 4 batch rows per core, embed table replicated.
  - Per (row, s-tile of 128 tokens) only the time band |t - c| <= R*sigma
    matters (Gaussian tails underflow); bands are computed on host from the
    actual durations (union across the 8 cores, since SPMD shares one
    program) and baked into the instruction stream.  A new input signature
    recompiles (cached by band signature).
  - On-device per (row, s-tile): z = (t - c')/sig from a small iota row
    (band-local, per-partition scale/bias), z^2, then
    w = exp(-0.5 z^2 - log sig - log sqrt(2pi)) via ACT Exp with
    per-partition bias.  w lives [s partitions, t free] = matmul lhsT.
    The z/square work is spread across DVE/ACT/GPSIMD by a load planner.
  - Embedding gather on device via one-hot matmul: token ids are partition-
    broadcast, compared against a per-partition id column (is_equal), then
    onehot.T @ embed_aug on the PE (embed shipped with an appended ones
    column -> emb_g[s,256] = 1, so the main matmul's PSUM column 256 is the
    normalization denominator).
  - Main matmul per output t-tile accumulates only contributing s-tiles.
    Normalization: recip(denom + eps), then PSUM->SBUF bf16 copies scaled by
    the per-frame reciprocal (batched tensor_tensor on DVE / activation on
    ACT, planner-balanced), one DMA per 4 t-tiles.
  - Frames t >= total duration are zeroed on host (reference semantics).
"""

import numpy as np

# Problem constants (kernel.py is self-contained; shapes hardcoded).
B, S, IDIM, D, T = 32, 512, 256, 256, 4096
EPS = 1e-6
SIGMA_C = 2.0
PAD = 0
LOG_SQRT_2PI = 0.9189385332046727
N_CORES = 8
NR = B // N_CORES          # rows per core
NK = S // 128              # s-tiles per row
R_BAND = 5.0               # Gaussian cutoff in sigmas

# Engine assignment config (tuned against TimelineSim):
#   w_mode per (r,k): 'A' = ACT Square; 'P' = GPSIMD z + GPSIMD square;
#     'D' = DVE z + GPSIMD square; 'V' = DVE z + DVE square;
#     'G' = GPSIMD z + DVE square.
#   nc_pat: normcopy engine cycle ('V' = DVE batched TT, 'A' = ACT per-tile)
#   ev_pat: emb_g evacuation engine cycle ('V'/'A')
#   oh_pat: one-hot is_equal engine cycle ('V'/'P')
#   bcast: 'pe' = ones-matmul broadcast, 'gp' = gpsimd partition_broadcast
CONFIG = {
    "w_mode": "VGVG" "AGVG" "VGVG" "AGVG",
    "nc_pat": "VVA",
    "ev_pat": "A",
    "oh_pat": "P",
    "bcast": "pe",
    "grp": 4,     # output t-tiles per PSUM group
    "psbufs": 3,  # PSUM group slots in flight
    "wbufs": 12,  # working sbuf pool bufs
}

_runners = {}


def _host_prep(text, durs):
    durs = np.asarray(durs).astype(np.int64)
    text = np.asarray(text).astype(np.int64)
    excl = np.cumsum(durs, axis=-1) - durs
    L = np.minimum(np.cumsum(durs, axis=-1)[:, -1], T).astype(np.int64)
    durs_f = durs.astype(np.float64)
    c = durs_f / 2.0 + excl.astype(np.float64)
    sig = durs_f / SIGMA_C + EPS
    valid = (durs > 0) & (text != PAD)

    inv_sig = np.where(valid, 1.0 / sig, 1e6)
    cprime = np.where(valid, c - 0.5, -4e6)
    b2 = np.where(valid, -np.log(sig) - LOG_SQRT_2PI, 0.0)

    bands = {}
    for r in range(NR):
        for k in range(NK):
            lo, hi = float(T), 0.0
            for cid in range(N_CORES):
                b = cid * NR + r
                sl = slice(k * 128, (k + 1) * 128)
                m = valid[b, sl]
                if not m.any():
                    continue
                cs, ss = c[b, sl][m], sig[b, sl][m]
                lo = min(lo, (cs - R_BAND * ss).min())
                hi = max(hi, (cs + R_BAND * ss).max())
            if hi <= lo:
                bands[(r, k)] = None
                continue
            lo = int(max(0, np.floor(lo / 128.0) * 128))
            hi = int(min(T, np.ceil(hi / 128.0) * 128))
            bands[(r, k)] = (lo, hi)
    ncov = []
    for r in range(NR):
        hi_r = max((bands[(r, k)][1] for k in range(NK) if bands[(r, k)]),
                   default=0)
        ncov.append(hi_r // 128)

    # Safety: every frame t < L_b must be covered by some band of its row.
    for b in range(B):
        r = b % NR
        need = min(int(np.ceil(L[b] / 128.0)), T // 128)
        for tt in range(need):
            ok = any(
                bands[(r, k)] and bands[(r, k)][0] < (tt + 1) * 128
                and bands[(r, k)][1] > tt * 128
                for k in range(NK)
            )
            if not ok:  # pragma: no cover - fall back to dense bands
                for k in range(NK):
                    bands[(r, k)] = (0, T)
                ncov[r] = T // 128
                break

    params = np.zeros((N_CORES, 128, NR * NK * 4), dtype=np.float32)
    textf = np.zeros((N_CORES, NR, S), dtype=np.float32)
    for cid in range(N_CORES):
        for r in range(NR):
            b = cid * NR + r
            textf[cid, r] = text[b].astype(np.float32)
            for k in range(NK):
                if bands[(r, k)] is None:
                    continue
                lo = bands[(r, k)][0]
                sl = slice(k * 128, (k + 1) * 128)
                j = (r * NK + k) * 4
                cp = cprime[b, sl] - lo      # band-local center
                params[cid, :, j + 0] = cp
                params[cid, :, j + 1] = inv_sig[b, sl]
                params[cid, :, j + 2] = -cp * inv_sig[b, sl]
                params[cid, :, j + 3] = b2[b, sl]
    return params, textf, bands, ncov, L


def _build_program(bands, ncov, reps=1, loops=0):
    import concourse.bacc as bacc
    import concourse.mybir as mybir
    import concourse.tile as tile

    dt = mybir.dt

    nc = bacc.Bacc("TRN2", target_bir_lowering=False, debug=False,
                   num_devices=N_CORES)
    d_params = nc.dram_tensor("params", [128, NR * NK * 4], dt.float32,
                              kind="ExternalInput").ap()
    d_idcol = nc.dram_tensor("idcol", [128, 1], dt.float32,
                             kind="ExternalInput").ap()
    d_textf = nc.dram_tensor("textf", [NR, S], dt.float32,
                             kind="ExternalInput").ap()
    d_embaug = nc.dram_tensor("embaug", [IDIM, D], dt.bfloat16,
                              kind="ExternalInput").ap()
    d_out = nc.dram_tensor("out", [NR, T, D], dt.bfloat16,
                           kind="ExternalOutput").ap()

    wmax = max((hi - lo) for v in bands.values() if v for lo, hi in [v])

    with tile.TileContext(nc) as tc:
        with tc.tile_pool(name="const", bufs=1) as cpool:
            iota_t = cpool.tile([128, wmax], dt.float32, tag="iota")
            nc.gpsimd.iota(iota_t[:, :], pattern=[[1, wmax]], base=0,
                           channel_multiplier=0,
                           allow_small_or_imprecise_dtypes=True)
            params_t = cpool.tile([128, NR * NK * 4], dt.float32, tag="par")
            nc.sync.dma_start(out=params_t[:, :], in_=d_params[:, :])
            idcol_t = cpool.tile([128, 1], dt.float32, tag="idc")
            nc.sync.dma_start(out=idcol_t[:, :], in_=d_idcol[:, :])
            embaug_t = cpool.tile([128, 2, D], dt.bfloat16, tag="emb")
            nc.sync.dma_start(
                out=embaug_t[:, :, :],
                in_=d_embaug.rearrange("(c p) n -> p c n", p=128))
            ones1 = cpool.tile([1, 128], dt.float32, tag="ones")
            nc.gpsimd.memset(ones1[:, :], 1.0)
            onesb = cpool.tile([128, 1], dt.bfloat16, tag="onesb")
            nc.gpsimd.memset(onesb[:, :], 1.0)

            if loops:
                # timing variant: body repeated `loops` times on device,
                # writing internal DRAM scratch; tiny external output.
                d_s = nc.dram_tensor("scratch_out", [NR, T, D],
                                     dt.bfloat16).ap()
                with tc.For_i(0, loops, 1):
                    _emit_body(nc, tc, bands, ncov, d_textf, d_s, iota_t,
                               params_t, idcol_t, embaug_t, ones1, onesb, 0)
                fin = cpool.tile([128, 16], dt.bfloat16, tag="fin")
                nc.vector.memset(fin[:, :], 1.0)
                nc.sync.dma_start(out=d_out[0, 0:128, 0:16], in_=fin[:, :])
            else:
                for rep in range(reps):
                    d_o = d_out if rep == 0 else nc.dram_tensor(
                        f"scratch_out{rep}", [NR, T, D], dt.bfloat16).ap()
                    _emit_body(nc, tc, bands, ncov, d_textf, d_o, iota_t,
                               params_t, idcol_t, embaug_t, ones1, onesb,
                               rep)
    nc.compile()
    return nc


def _emit_body(nc, tc, bands, ncov, d_textf, d_out, iota_t, params_t,
               idcol_t, embaug_t, ones1, onesb, rep):
    from contextlib import ExitStack
    import concourse.mybir as mybir

    dt = mybir.dt
    Act = mybir.ActivationFunctionType
    Alu = mybir.AluOpType
    cfg = CONFIG
    nci = [0]
    evi = [0]
    ohi = [0]

    def cyc(pat, i):
        c = pat[i[0] % len(pat)]
        i[0] += 1
        return c

    with ExitStack() as ctx:
        embg_pool = ctx.enter_context(
            tc.tile_pool(name=f"embg{rep}", bufs=NR * NK))
        sbuf = ctx.enter_context(
            tc.tile_pool(name=f"work{rep}", bufs=cfg["wbufs"]))
        wpool = ctx.enter_context(tc.tile_pool(name=f"wt{rep}", bufs=2 * NK))
        psum = ctx.enter_context(
            tc.tile_pool(name=f"ps{rep}", bufs=cfg["psbufs"], space="PSUM"))
        psum_sm = ctx.enter_context(
            tc.tile_pool(name=f"pss{rep}", bufs=2, space="PSUM"))

        embg = {}
        wt = {}
        for r in range(NR):
            # ---- gather: one-hot matmul -> emb_g[r][k] ----
            trow = sbuf.tile([1, S], dt.float32, tag="trow")
            nc.sync.dma_start(out=trow[:, :], in_=d_textf[r:r + 1, :])
            if cfg["bcast"] == "pe":
                tbp = psum.tile([128, S], dt.float32, tag="big")
                nc.tensor.matmul(tbp[:, :], ones1[:, :], trow[:, :],
                                 start=True, stop=True)
                tb = tbp
            else:
                tb = sbuf.tile([128, S], dt.float32, tag="tb")
                nc.gpsimd.partition_broadcast(tb[:, :], trow[:, :])
            oh = []
            for cch in range(IDIM // 128):
                o = sbuf.tile([128, S], dt.bfloat16, tag="oh")
                e = cyc(cfg["oh_pat"], ohi)
                # GPSIMD cannot read PSUM; PE-broadcast tb lives in PSUM
                eng = nc.vector if (e == "V" or cfg["bcast"] == "pe") else nc.gpsimd
                eng.tensor_scalar(
                    o[:, :], tb[:, :], idcol_t[:, 0:1], float(128 * cch),
                    Alu.subtract, Alu.is_equal)
                oh.append(o)
            for k in range(NK):
                eg = psum_sm.tile([128, D], dt.float32, tag="egden")
                for cch in range(IDIM // 128):
                    nc.tensor.matmul(
                        eg[:, :], oh[cch][:, k * 128:(k + 1) * 128],
                        embaug_t[:, cch, :],
                        start=(cch == 0), stop=(cch == 1))
                g = embg_pool.tile([128, D], dt.bfloat16, tag="embg")
                if cyc(cfg["ev_pat"], evi) == "V":
                    nc.vector.tensor_copy(g[:, :], eg[:, :])
                else:
                    nc.scalar.copy(g[:, :], eg[:, :])
                embg[(r, k)] = g

            # ---- w tiles for this row ----
            for k in range(NK):
                if bands[(r, k)] is None:
                    continue
                lo, hi = bands[(r, k)]
                W = hi - lo
                j = (r * NK + k) * 4
                mode = cfg["w_mode"][r * NK + k]
                sq = sbuf.tile([128, W], dt.bfloat16, tag="sq")
                if mode == "A":
                    nc.scalar.activation(sq[:, :], iota_t[:, 0:W],
                                         Act.Square,
                                         bias=params_t[:, j + 2:j + 3],
                                         scale=params_t[:, j + 1:j + 2])
                else:
                    z = sbuf.tile([128, W], dt.bfloat16, tag="z")
                    zeng = nc.gpsimd if mode in "PG" else nc.vector
                    seng = nc.vector if mode in "VG" else nc.gpsimd
                    zeng.tensor_scalar(z[:, :], iota_t[:, 0:W],
                                       params_t[:, j + 0:j + 1],
                                       params_t[:, j + 1:j + 2],
                                       Alu.subtract, Alu.mult)
                    seng.tensor_tensor(sq[:, :], z[:, :], z[:, :], Alu.mult)
                w = wpool.tile([128, W], dt.bfloat16, tag="w")
                nc.scalar.activation(w[:, :], sq[:, :], Act.Exp,
                                     bias=params_t[:, j + 3:j + 4],
                                     scale=-0.5)
                wt[(r, k)] = (w, lo, hi)

            # ---- main matmuls + normalize + out DMA ----
            GRP = cfg["grp"]
            for g0 in range(0, ncov[r], GRP):
                g = min(GRP, ncov[r] - g0)
                big = psum.tile([128, GRP * D], dt.float32, tag="big")
                den = psum_sm.tile([128, 8], dt.float32, tag="egden")
                for q in range(g):
                    tt = g0 + q
                    ks = [k for k in range(NK)
                          if bands[(r, k)]
                          and bands[(r, k)][0] < (tt + 1) * 128
                          and bands[(r, k)][1] > tt * 128]
                    dst = big[:, q * D:(q + 1) * D]
                    for ji, k in enumerate(ks):
                        w, lo, hi = wt[(r, k)]
                        off = tt * 128 - lo
                        st, sp = (ji == 0), (ji == len(ks) - 1)
                        nc.tensor.matmul(dst, w[:, off:off + 128],
                                         embg[(r, k)][:, :],
                                         start=st, stop=sp)
                        nc.tensor.matmul(den[:, q:q + 1], w[:, off:off + 128],
                                         onesb[:, :], start=st, stop=sp)
                rec = sbuf.tile([128, 8], dt.float32, tag="rec")
                nc.vector.reciprocal(rec[:, 0:g], den[:, 0:g])
                stage = sbuf.tile([128, GRP * D], dt.bfloat16, tag="stage")
                if cyc(cfg["nc_pat"], nci) == "V":
                    nc.vector.tensor_tensor(
                        stage[:, 0:g * D].rearrange("p (g d) -> p g d", g=g),
                        big[:, 0:g * D].rearrange("p (g d) -> p g d", g=g),
                        rec[:, 0:g, None].broadcast_to([128, g, D]),
                        Alu.mult)
                else:
                    for q in range(g):
                        nc.scalar.mul(stage[:, q * D:(q + 1) * D],
                                      big[:, q * D:(q + 1) * D],
                                      rec[:, q:q + 1])
                nc.sync.dma_start(
                    out=d_out[r, g0 * 128:(g0 + g) * 128, :].rearrange(
                        "(q p) d -> p q d", p=128),
                    in_=stage[:, 0:g * D])


def _make_runner(nc):
    import jax
    import concourse.mybir as mybir
    from concourse import bass2jax
    from jax.experimental.shard_map import shard_map
    from jax.sharding import Mesh, PartitionSpec

    bass2jax.install_neuronx_cc_hook()

    partition_name = (nc.partition_id_tensor.name
                      if nc.partition_id_tensor else None)
    in_names, out_names, out_avals, out_shapes = [], [], [], []
    for alloc in nc.m.functions[0].allocations:
        if not isinstance(alloc, mybir.MemoryLocationSet):
            continue
        name = alloc.memorylocations[0].name
        if alloc.kind == "ExternalInput":
            if name != partition_name:
                in_names.append(name)
        elif alloc.kind == "ExternalOutput":
            out_names.append(name)
            shape = tuple(alloc.tensor_shape)
            dtype = mybir.dt.np(alloc.dtype)
            out_avals.append(jax.core.ShapedArray(shape, dtype))
            out_shapes.append((shape, dtype))
    n_params = len(in_names)
    all_in_names = list(in_names) + list(out_names)
    if partition_name is not None:
        all_in_names.append(partition_name)
    all_in_names = tuple(all_in_names)
    donate = tuple(range(n_params, n_params + len(out_names)))

    def _body(*args):
        operands = list(args)
        if partition_name is not None:
            operands.append(bass2jax.partition_id_tensor())
        outs = bass2jax._bass_exec_p.bind(
            *operands,
            out_avals=tuple(out_avals),
            in_names=all_in_names,
            out_names=tuple(out_names),
            lowering_input_output_aliases=(),
            sim_require_finite=True,
            sim_require_nnan=True,
            nc=nc,
        )
        return tuple(outs)

    devices = jax.devices()[:N_CORES]
    mesh = Mesh(np.asarray(devices), ("core",))
    specs = (PartitionSpec("core"),) * (n_params + len(out_names))
    out_specs = (PartitionSpec("core"),) * len(out_names)
    fn = jax.jit(
        shard_map(_body, mesh=mesh, in_specs=specs, out_specs=out_specs,
                  check_rep=False),
        donate_argnums=donate, keep_unused=True)

    state = {"prev_outs": None}

    def run(in_maps):
        concat_in = [
            np.concatenate([np.asarray(in_maps[c][n]) for c in range(N_CORES)],
                           axis=0)
            for n in in_names
        ]
        if state["prev_outs"] is not None:
            out_args = state["prev_outs"]
        else:
            out_args = [
                np.zeros((N_CORES * sh[0], *sh[1:]), dtp)
                for (sh, dtp) in out_shapes
            ]
        out_arrs = fn(*concat_in, *out_args)
        result = {
            name: np.asarray(out_arrs[i]).reshape(N_CORES, *out_shapes[i][0])
            for i, name in enumerate(out_names)
        }
        # donate this call's device-resident outputs back as next call's
        # output buffers (avoids re-uploading zeros through the tunnel)
        state["prev_outs"] = list(out_arrs)
        return result

    return run


def get_runner(key, bands, ncov, reps=1, loops=0):
    k = (key, reps, loops)
    if k not in _runners:
        nc = _build_program(bands, ncov, reps=reps, loops=loops)
        _runners[k] = _make_runner(nc)
    return _runners[k]


def kernel(text, durs, embed, total_time):
    import ml_dtypes

    text = np.asarray(text)
    durs = np.asarray(durs)
    embed = np.asarray(embed, dtype=np.float32)
    tt_ = int(np.asarray(total_time))
    assert tt_ == T and text.shape == (B, S) and embed.shape == (IDIM, D)

    params, textf, bands, ncov, L = _host_prep(text, durs)
    key = tuple(sorted((rk, v) for rk, v in bands.items())) + tuple(ncov)
    run = get_runner(key, bands, ncov)

    embaug = embed.astype(ml_dtypes.bfloat16)
    embaug[PAD, :] = 0
    idcol = np.arange(128, dtype=np.float32).reshape(128, 1)

    in_maps = [
        {"params": params[cid], "idcol": idcol, "textf": textf[cid],
         "embaug": embaug}
        for cid in range(N_CORES)
    ]
    res = run(in_maps)
    out = res["out"].astype(np.float32).reshape(B, T, D)
    for b in range(B):
        out[b, L[b]:, :] = 0.0
    return out


# revision 23
# speedup vs baseline: 2754.5298x; 3.1876x over previous
"""GaussianEmbedding Trainium2 Bass kernel (8-core data parallel).

out[b,t,:] = sum_s w[b,t,s] * embed[text[b,s]],  w = normalized Gaussian
weights centered at token centers c_s with sigma = dur_s/2.

Strategy:
  - Pure data parallel: 4 batch rows per core, embed table replicated.
  - Per (row, s-tile of 128 tokens) only the time band |t - c| <= R*sigma
    matters (Gaussian tails underflow); bands are computed on host from the
    actual durations (union across the 8 cores, since SPMD shares one
    program) and baked into the instruction stream.  A new input signature
    recompiles (cached by band signature).
  - On-device per (row, s-tile): z = (t - c')/sig from a small iota row
    (band-local, per-partition scale/bias), z^2, then
    w = exp(-0.5 z^2 - log sig - log sqrt(2pi)) via ACT Exp with
    per-partition bias.  w lives [s partitions, t free] = matmul lhsT.
    The z/square work is spread across DVE/ACT/GPSIMD by a load planner.
  - Embedding gather on device via one-hot matmul: token ids are partition-
    broadcast, compared against a per-partition id column (is_equal), then
    onehot.T @ embed_aug on the PE (embed shipped with an appended ones
    column -> emb_g[s,256] = 1, so the main matmul's PSUM column 256 is the
    normalization denominator).
  - Main matmul per output t-tile accumulates only contributing s-tiles.
    Normalization: recip(denom + eps), then PSUM->SBUF bf16 copies scaled by
    the per-frame reciprocal (batched tensor_tensor on DVE / activation on
    ACT, planner-balanced), one DMA per 4 t-tiles.
  - Frames t >= total duration are zeroed on host (reference semantics).
"""

import numpy as np

# Problem constants (kernel.py is self-contained; shapes hardcoded).
B, S, IDIM, D, T = 32, 512, 256, 256, 4096
EPS = 1e-6
SIGMA_C = 2.0
PAD = 0
LOG_SQRT_2PI = 0.9189385332046727
N_CORES = 8
NR = B // N_CORES          # rows per core
NK = S // 128              # s-tiles per row
R_BAND = 5.0               # Gaussian cutoff in sigmas

# Engine assignment config (tuned against TimelineSim):
#   w_mode per (r,k): 'A' = ACT Square; 'P' = GPSIMD z + GPSIMD square;
#     'D' = DVE z + GPSIMD square; 'V' = DVE z + DVE square;
#     'G' = GPSIMD z + DVE square.
#   nc_pat: normcopy engine cycle ('V' = DVE batched TT, 'A' = ACT per-tile)
#   ev_pat: emb_g evacuation engine cycle ('V'/'A')
#   oh_pat: one-hot is_equal engine cycle ('V'/'P')
#   bcast: 'pe' = ones-matmul broadcast, 'gp' = gpsimd partition_broadcast
CONFIG = {
    "w_mode": "A" * 16,
    "nc_pat": "V",
    "ev_pat": "V",
    "oh_pat": "V",
    "bcast": "pe",
    "grp": 3,     # output t-tiles per PSUM group
    "psbufs": 2,  # PSUM group slots in flight
    "wbufs": 12,  # working sbuf pool bufs
    "ablate": "full",
}

_runners = {}


def _host_prep(text, durs):
    durs = np.asarray(durs).astype(np.int64)
    text = np.asarray(text).astype(np.int64)
    excl = np.cumsum(durs, axis=-1) - durs
    L = np.minimum(np.cumsum(durs, axis=-1)[:, -1], T).astype(np.int64)
    durs_f = durs.astype(np.float64)
    c = durs_f / 2.0 + excl.astype(np.float64)
    sig = durs_f / SIGMA_C + EPS
    valid = (durs > 0) & (text != PAD)

    inv_sig = np.where(valid, 1.0 / sig, 1e6)
    cprime = np.where(valid, c - 0.5, -4e6)
    b2 = np.where(valid, -np.log(sig) - LOG_SQRT_2PI, 0.0)

    bands = {}
    for r in range(NR):
        for k in range(NK):
            lo, hi = float(T), 0.0
            for cid in range(N_CORES):
                b = cid * NR + r
                sl = slice(k * 128, (k + 1) * 128)
                m = valid[b, sl]
                if not m.any():
                    continue
                cs, ss = c[b, sl][m], sig[b, sl][m]
                lo = min(lo, (cs - R_BAND * ss).min())
                hi = max(hi, (cs + R_BAND * ss).max())
            if hi <= lo:
                bands[(r, k)] = None
                continue
            lo = int(max(0, np.floor(lo / 128.0) * 128))
            hi = int(min(T, np.ceil(hi / 128.0) * 128))
            bands[(r, k)] = (lo, hi)
    ncov = []
    for r in range(NR):
        hi_r = max((bands[(r, k)][1] for k in range(NK) if bands[(r, k)]),
                   default=0)
        ncov.append(hi_r // 128)

    # Safety: every frame t < L_b must be covered by some band of its row.
    for b in range(B):
        r = b % NR
        need = min(int(np.ceil(L[b] / 128.0)), T // 128)
        for tt in range(need):
            ok = any(
                bands[(r, k)] and bands[(r, k)][0] < (tt + 1) * 128
                and bands[(r, k)][1] > tt * 128
                for k in range(NK)
            )
            if not ok:  # pragma: no cover - fall back to dense bands
                for k in range(NK):
                    bands[(r, k)] = (0, T)
                ncov[r] = T // 128
                break

    params = np.zeros((N_CORES, 128, NR * NK * 4), dtype=np.float32)
    textf = np.zeros((N_CORES, NR, S), dtype=np.float32)
    for cid in range(N_CORES):
        for r in range(NR):
            b = cid * NR + r
            textf[cid, r] = text[b].astype(np.float32)
            for k in range(NK):
                if bands[(r, k)] is None:
                    continue
                lo = bands[(r, k)][0]
                sl = slice(k * 128, (k + 1) * 128)
                j = (r * NK + k) * 4
                cp = cprime[b, sl] - lo      # band-local center
                params[cid, :, j + 0] = cp
                params[cid, :, j + 1] = inv_sig[b, sl]
                params[cid, :, j + 2] = -cp * inv_sig[b, sl]
                params[cid, :, j + 3] = b2[b, sl]
    return params, textf, bands, ncov, L


def _build_program(bands, ncov, reps=1, loops=0):
    import concourse.bacc as bacc
    import concourse.mybir as mybir
    import concourse.tile as tile

    dt = mybir.dt

    nc = bacc.Bacc("TRN2", target_bir_lowering=False, debug=False,
                   num_devices=N_CORES)
    d_params = nc.dram_tensor("params", [128, NR * NK * 4], dt.float32,
                              kind="ExternalInput").ap()
    d_idcol = nc.dram_tensor("idcol", [128, 1], dt.float32,
                             kind="ExternalInput").ap()
    d_textf = nc.dram_tensor("textf", [NR, S], dt.float32,
                             kind="ExternalInput").ap()
    d_embaug = nc.dram_tensor("embaug", [IDIM, D + 1], dt.bfloat16,
                              kind="ExternalInput").ap()
    if loops:
        d_out = nc.dram_tensor("out", [128, 16], dt.bfloat16,
                               kind="ExternalOutput").ap()
    else:
        d_out = nc.dram_tensor("out", [NR, T, D], dt.bfloat16,
                               kind="ExternalOutput").ap()

    wmax = max((hi - lo) for v in bands.values() if v for lo, hi in [v])

    with tile.TileContext(nc) as tc:
        with tc.tile_pool(name="const", bufs=1) as cpool:
            iota_t = cpool.tile([128, wmax], dt.float32, tag="iota")
            nc.gpsimd.iota(iota_t[:, :], pattern=[[1, wmax]], base=0,
                           channel_multiplier=0,
                           allow_small_or_imprecise_dtypes=True)
            params_t = cpool.tile([128, NR * NK * 4], dt.float32, tag="par")
            nc.sync.dma_start(out=params_t[:, :], in_=d_params[:, :])
            idcol_t = cpool.tile([128, 1], dt.float32, tag="idc")
            nc.sync.dma_start(out=idcol_t[:, :], in_=d_idcol[:, :])
            embaug_t = cpool.tile([128, 2, D + 1], dt.bfloat16, tag="emb")
            nc.sync.dma_start(
                out=embaug_t[:, :, :],
                in_=d_embaug.rearrange("(c p) n -> p c n", p=128))
            ones1 = cpool.tile([1, 128], dt.float32, tag="ones")
            nc.gpsimd.memset(ones1[:, :], 1.0)
            onesb = cpool.tile([128, 1], dt.bfloat16, tag="onesb")
            nc.gpsimd.memset(onesb[:, :], 1.0)

            if loops:
                # timing variant: body repeated `loops` times on device,
                # writing internal DRAM scratch; tiny external output.
                d_s = nc.dram_tensor("scratch_out", [NR, T, D],
                                     dt.bfloat16).ap()
                with tc.For_i(0, loops, 1):
                    _emit_body(nc, tc, bands, ncov, d_textf, d_s, iota_t,
                               params_t, idcol_t, embaug_t, ones1, onesb, 0)
                fin = cpool.tile([128, 16], dt.bfloat16, tag="fin")
                nc.vector.memset(fin[:, :], 1.0)
                nc.sync.dma_start(out=d_out[:, :], in_=fin[:, :])
            else:
                for rep in range(reps):
                    d_o = d_out if rep == 0 else nc.dram_tensor(
                        f"scratch_out{rep}", [NR, T, D], dt.bfloat16).ap()
                    _emit_body(nc, tc, bands, ncov, d_textf, d_o, iota_t,
                               params_t, idcol_t, embaug_t, ones1, onesb,
                               rep)
    nc.compile()
    return nc


def _emit_body(nc, tc, bands, ncov, d_textf, d_out, iota_t, params_t,
               idcol_t, embaug_t, ones1, onesb, rep):
    from contextlib import ExitStack
    import concourse.mybir as mybir

    dt = mybir.dt
    Act = mybir.ActivationFunctionType
    Alu = mybir.AluOpType
    cfg = CONFIG
    abl = cfg.get("ablate", "full")
    nci = [0]
    evi = [0]
    ohi = [0]

    def cyc(pat, i):
        c = pat[i[0] % len(pat)]
        i[0] += 1
        return c

    with ExitStack() as ctx:
        embg_pool = ctx.enter_context(
            tc.tile_pool(name=f"embg{rep}", bufs=NR * NK))
        sbuf = ctx.enter_context(
            tc.tile_pool(name=f"work{rep}", bufs=cfg["wbufs"]))
        wpool = ctx.enter_context(tc.tile_pool(name=f"wt{rep}", bufs=2 * NK))
        psum = ctx.enter_context(
            tc.tile_pool(name=f"ps{rep}", bufs=cfg["psbufs"], space="PSUM"))
        psum_sm = ctx.enter_context(
            tc.tile_pool(name=f"pss{rep}", bufs=2, space="PSUM"))

        embg = {}
        wt = {}
        for r in range(NR):
            if abl == "none":
                break
            # ---- gather: one-hot matmul -> emb_g[r][k] ----
            if abl not in ("wonly",):
                trow = sbuf.tile([1, S], dt.float32, tag="trow")
                nc.sync.dma_start(out=trow[:, :], in_=d_textf[r:r + 1, :])
                if cfg["bcast"] == "pe":
                    tbp = psum.tile([128, S], dt.float32, tag="big")
                    nc.tensor.matmul(tbp[:, :], ones1[:, :], trow[:, :],
                                     start=True, stop=True)
                    tb = tbp
                else:
                    tb = sbuf.tile([128, S], dt.float32, tag="tb")
                    nc.gpsimd.partition_broadcast(tb[:, :], trow[:, :])
                oh = []
                for cch in range(IDIM // 128):
                    o = sbuf.tile([128, S], dt.bfloat16, tag="oh")
                    e = cyc(cfg["oh_pat"], ohi)
                    # GPSIMD cannot read PSUM (PE-broadcast tb is PSUM)
                    eng = (nc.vector
                           if (e == "V" or cfg["bcast"] == "pe")
                           else nc.gpsimd)
                    eng.tensor_scalar(
                        o[:, :], tb[:, :], idcol_t[:, 0:1], float(128 * cch),
                        Alu.subtract, Alu.is_equal)
                    oh.append(o)
                for k in range(NK):
                    eg = psum_sm.tile([128, D + 1], dt.float32, tag="egden")
                    for cch in range(IDIM // 128):
                        nc.tensor.matmul(
                            eg[:, :], oh[cch][:, k * 128:(k + 1) * 128],
                            embaug_t[:, cch, :],
                            start=(cch == 0), stop=(cch == 1))
                    g = embg_pool.tile([128, D + 1], dt.bfloat16, tag="embg")
                    if cyc(cfg["ev_pat"], evi) == "V":
                        nc.vector.tensor_copy(g[:, :], eg[:, :])
                    else:
                        nc.scalar.copy(g[:, :], eg[:, :])
                    embg[(r, k)] = g

            # ---- w tiles for this row ----
            for k in range(NK):
                if bands[(r, k)] is None:
                    continue
                lo, hi = bands[(r, k)]
                W = hi - lo
                j = (r * NK + k) * 4
                mode = cfg["w_mode"][r * NK + k]
                sq = sbuf.tile([128, W], dt.bfloat16, tag="sq")
                if mode == "A":
                    nc.scalar.activation(sq[:, :], iota_t[:, 0:W],
                                         Act.Square,
                                         bias=params_t[:, j + 2:j + 3],
                                         scale=params_t[:, j + 1:j + 2])
                else:
                    z = sbuf.tile([128, W], dt.bfloat16, tag="z")
                    zeng = nc.gpsimd if mode in "PG" else nc.vector
                    seng = nc.vector if mode in "VG" else nc.gpsimd
                    zeng.tensor_scalar(z[:, :], iota_t[:, 0:W],
                                       params_t[:, j + 0:j + 1],
                                       params_t[:, j + 1:j + 2],
                                       Alu.subtract, Alu.mult)
                    seng.tensor_tensor(sq[:, :], z[:, :], z[:, :], Alu.mult)
                w = wpool.tile([128, W], dt.bfloat16, tag="w")
                nc.scalar.activation(w[:, :], sq[:, :], Act.Exp,
                                     bias=params_t[:, j + 3:j + 4],
                                     scale=-0.5)
                wt[(r, k)] = (w, lo, hi)

            if abl in ("wonly", "nomm"):
                continue

            # ---- main matmuls + normalize + out DMA ----
            GRP = cfg["grp"]
            for g0 in range(0, ncov[r], GRP):
                g = min(GRP, ncov[r] - g0)
                big = psum.tile([128, GRP * 512], dt.float32, tag="big")
                for q in range(g):
                    tt = g0 + q
                    ks = [k for k in range(NK)
                          if bands[(r, k)]
                          and bands[(r, k)][0] < (tt + 1) * 128
                          and bands[(r, k)][1] > tt * 128]
                    dst = big[:, q * 512:q * 512 + D + 1]
                    for ji, k in enumerate(ks):
                        w, lo, hi = wt[(r, k)]
                        off = tt * 128 - lo
                        nc.tensor.matmul(dst, w[:, off:off + 128],
                                         embg[(r, k)][:, :],
                                         start=(ji == 0),
                                         stop=(ji == len(ks) - 1))
                if abl == "nonorm":
                    continue
                rec = sbuf.tile([128, 8], dt.float32, tag="rec")
                nc.vector.reciprocal(
                    rec[:, 0:g], big[:, D:D + (g - 1) * 512 + 1:512])
                stage = sbuf.tile([128, GRP * D], dt.bfloat16, tag="stage")
                if cyc(cfg["nc_pat"], nci) == "V":
                    nc.vector.tensor_tensor(
                        stage[:, 0:g * D].rearrange("p (g d) -> p g d", g=g),
                        big[:, 0:g * 512].rearrange(
                            "p (g x) -> p g x", g=g)[:, :, 0:D],
                        rec[:, 0:g, None].broadcast_to([128, g, D]),
                        Alu.mult)
                else:
                    for q in range(g):
                        nc.scalar.mul(stage[:, q * D:(q + 1) * D],
                                      big[:, q * 512:q * 512 + D],
                                      rec[:, q:q + 1])
                nc.sync.dma_start(
                    out=d_out[r, g0 * 128:(g0 + g) * 128, :].rearrange(
                        "(q p) d -> p q d", p=128),
                    in_=stage[:, 0:g * D])


def _make_runner(nc):
    import jax
    import concourse.mybir as mybir
    from concourse import bass2jax
    from jax.experimental.shard_map import shard_map
    from jax.sharding import Mesh, PartitionSpec

    bass2jax.install_neuronx_cc_hook()

    partition_name = (nc.partition_id_tensor.name
                      if nc.partition_id_tensor else None)
    in_names, out_names, out_avals, out_shapes = [], [], [], []
    for alloc in nc.m.functions[0].allocations:
        if not isinstance(alloc, mybir.MemoryLocationSet):
            continue
        name = alloc.memorylocations[0].name
        if alloc.kind == "ExternalInput":
            if name != partition_name:
                in_names.append(name)
        elif alloc.kind == "ExternalOutput":
            out_names.append(name)
            shape = tuple(alloc.tensor_shape)
            dtype = mybir.dt.np(alloc.dtype)
            out_avals.append(jax.core.ShapedArray(shape, dtype))
            out_shapes.append((shape, dtype))
    n_params = len(in_names)
    all_in_names = list(in_names) + list(out_names)
    if partition_name is not None:
        all_in_names.append(partition_name)
    all_in_names = tuple(all_in_names)
    donate = tuple(range(n_params, n_params + len(out_names)))

    def _body(*args):
        operands = list(args)
        if partition_name is not None:
            operands.append(bass2jax.partition_id_tensor())
        outs = bass2jax._bass_exec_p.bind(
            *operands,
            out_avals=tuple(out_avals),
            in_names=all_in_names,
            out_names=tuple(out_names),
            lowering_input_output_aliases=(),
            sim_require_finite=True,
            sim_require_nnan=True,
            nc=nc,
        )
        return tuple(outs)

    devices = jax.devices()[:N_CORES]
    mesh = Mesh(np.asarray(devices), ("core",))
    specs = (PartitionSpec("core"),) * (n_params + len(out_names))
    out_specs = (PartitionSpec("core"),) * len(out_names)
    fn = jax.jit(
        shard_map(_body, mesh=mesh, in_specs=specs, out_specs=out_specs,
                  check_rep=False),
        donate_argnums=donate, keep_unused=True)

    state = {"prev_outs": None}

    def run(in_maps):
        concat_in = [
            np.concatenate([np.asarray(in_maps[c][n]) for c in range(N_CORES)],
                           axis=0)
            for n in in_names
        ]
        if state["prev_outs"] is not None:
            out_args = state["prev_outs"]
        else:
            out_args = [
                np.zeros((N_CORES * sh[0], *sh[1:]), dtp)
                for (sh, dtp) in out_shapes
            ]
        out_arrs = fn(*concat_in, *out_args)
        result = {
            name: np.asarray(out_arrs[i]).reshape(N_CORES, *out_shapes[i][0])
            for i, name in enumerate(out_names)
        }
        # donate this call's device-resident outputs back as next call's
        # output buffers (avoids re-uploading zeros through the tunnel)
        state["prev_outs"] = list(out_arrs)
        return result

    return run


def get_runner(key, bands, ncov, reps=1, loops=0):
    k = (key, reps, loops)
    if k not in _runners:
        nc = _build_program(bands, ncov, reps=reps, loops=loops)
        _runners[k] = _make_runner(nc)
    return _runners[k]


def kernel(text, durs, embed, total_time):
    import ml_dtypes

    text = np.asarray(text)
    durs = np.asarray(durs)
    embed = np.asarray(embed, dtype=np.float32)
    tt_ = int(np.asarray(total_time))
    assert tt_ == T and text.shape == (B, S) and embed.shape == (IDIM, D)

    params, textf, bands, ncov, L = _host_prep(text, durs)
    key = tuple(sorted((rk, v) for rk, v in bands.items())) + tuple(ncov)
    run = get_runner(key, bands, ncov)

    embaug = np.ones((IDIM, D + 1), dtype=ml_dtypes.bfloat16)
    embaug[:, :D] = embed.astype(ml_dtypes.bfloat16)
    embaug[PAD, :D] = 0
    idcol = np.arange(128, dtype=np.float32).reshape(128, 1)

    in_maps = [
        {"params": params[cid], "idcol": idcol, "textf": textf[cid],
         "embaug": embaug}
        for cid in range(N_CORES)
    ]
    res = run(in_maps)
    out = res["out"].astype(np.float32).reshape(B, T, D)
    for b in range(B):
        out[b, L[b]:, :] = 0.0
    return out


# revision 24
# speedup vs baseline: 3303.5275x; 1.1993x over previous
"""GaussianEmbedding Trainium2 Bass kernel (8-core data parallel).

out[b,t,:] = sum_s w[b,t,s] * embed[text[b,s]],  w = normalized Gaussian
weights centered at token centers c_s with sigma = dur_s/2.

Strategy:
  - Pure data parallel: 4 batch rows per core, embed table replicated.
  - Per (row, s-tile of 128 tokens) only the time band |t - c| <= R*sigma
    matters (Gaussian tails underflow); bands are computed on host from the
    actual durations (union across the 8 cores, since SPMD shares one
    program) and baked into the instruction stream.  A new input signature
    recompiles (cached by band signature).
  - On-device per (row, s-tile): z^2 = Square(iota*inv_sig + bias) in one
    ACT op (band-local iota, per-partition scale/bias APs), then
    w = exp(-0.5 z^2 - log sig - log sqrt(2pi)) via ACT Exp with
    per-partition bias.  w lives [s partitions, t free] = matmul lhsT.
    Engine assignment is configurable and was tuned on hardware: ACT small
    ops and GPSIMD element-wise ops are far more expensive on silicon than
    the cost model predicts, so everything PSUM-facing runs as fat batched
    DVE ops and the z/square work rides the two big ACT activations.
  - Embedding gather on device via one-hot matmul: token ids are partition-
    broadcast, compared against a per-partition id column (is_equal), then
    onehot.T @ embed_aug on the PE (embed shipped with an appended ones
    column -> emb_g[s,256] = 1, so the main matmul's PSUM column 256 is the
    normalization denominator).
  - Main matmul per output t-tile accumulates only contributing s-tiles.
    Normalization: per-frame reciprocal of the denominator column, then one
    batched PSUM->SBUF bf16 tensor_tensor per 3-t-tile group, one DMA per
    group.  (No +eps needed: any frame with denom == 0 lies beyond the
    row's total duration and is zeroed on host.)
  - Frames t >= total duration are zeroed on host (reference semantics).
"""

import numpy as np

# Problem constants (kernel.py is self-contained; shapes hardcoded).
B, S, IDIM, D, T = 32, 512, 256, 256, 4096
EPS = 1e-6
SIGMA_C = 2.0
PAD = 0
LOG_SQRT_2PI = 0.9189385332046727
N_CORES = 8
NR = B // N_CORES          # rows per core
NK = S // 128              # s-tiles per row
R_BAND = 5.0               # Gaussian cutoff in sigmas

# Engine assignment config (tuned against TimelineSim):
#   w_mode per (r,k): 'A' = ACT Square; 'P' = GPSIMD z + GPSIMD square;
#     'D' = DVE z + GPSIMD square; 'V' = DVE z + DVE square;
#     'G' = GPSIMD z + DVE square.
#   nc_pat: normcopy engine cycle ('V' = DVE batched TT, 'A' = ACT per-tile)
#   ev_pat: emb_g evacuation engine cycle ('V'/'A')
#   oh_pat: one-hot is_equal engine cycle ('V'/'P')
#   bcast: 'pe' = ones-matmul broadcast, 'gp' = gpsimd partition_broadcast
CONFIG = {
    "w_mode": "A" * 16,
    "nc_pat": "V",
    "ev_pat": "V",
    "oh_pat": "V",
    "bcast": "pe",
    "grp": 3,     # output t-tiles per PSUM group
    "psbufs": 2,  # PSUM group slots in flight
    "wbufs": 12,  # working sbuf pool bufs
    "ablate": "full",
}

_runners = {}


def _host_prep(text, durs):
    durs = np.asarray(durs).astype(np.int64)
    text = np.asarray(text).astype(np.int64)
    excl = np.cumsum(durs, axis=-1) - durs
    L = np.minimum(np.cumsum(durs, axis=-1)[:, -1], T).astype(np.int64)
    durs_f = durs.astype(np.float64)
    c = durs_f / 2.0 + excl.astype(np.float64)
    sig = durs_f / SIGMA_C + EPS
    valid = (durs > 0) & (text != PAD)

    inv_sig = np.where(valid, 1.0 / sig, 1e6)
    cprime = np.where(valid, c - 0.5, -4e6)
    b2 = np.where(valid, -np.log(sig) - LOG_SQRT_2PI, 0.0)

    bands = {}
    for r in range(NR):
        for k in range(NK):
            lo, hi = float(T), 0.0
            for cid in range(N_CORES):
                b = cid * NR + r
                sl = slice(k * 128, (k + 1) * 128)
                m = valid[b, sl]
                if not m.any():
                    continue
                cs, ss = c[b, sl][m], sig[b, sl][m]
                lo = min(lo, (cs - R_BAND * ss).min())
                hi = max(hi, (cs + R_BAND * ss).max())
            if hi <= lo:
                bands[(r, k)] = None
                continue
            lo = int(max(0, np.floor(lo / 128.0) * 128))
            hi = int(min(T, np.ceil(hi / 128.0) * 128))
            bands[(r, k)] = (lo, hi)
    ncov = []
    for r in range(NR):
        hi_r = max((bands[(r, k)][1] for k in range(NK) if bands[(r, k)]),
                   default=0)
        ncov.append(hi_r // 128)

    # Safety: every frame t < L_b must be covered by some band of its row.
    for b in range(B):
        r = b % NR
        need = min(int(np.ceil(L[b] / 128.0)), T // 128)
        for tt in range(need):
            ok = any(
                bands[(r, k)] and bands[(r, k)][0] < (tt + 1) * 128
                and bands[(r, k)][1] > tt * 128
                for k in range(NK)
            )
            if not ok:  # pragma: no cover - fall back to dense bands
                for k in range(NK):
                    bands[(r, k)] = (0, T)
                ncov[r] = T // 128
                break

    params = np.zeros((N_CORES, 128, NR * NK * 4), dtype=np.float32)
    textf = np.zeros((N_CORES, NR, S), dtype=np.float32)
    for cid in range(N_CORES):
        for r in range(NR):
            b = cid * NR + r
            textf[cid, r] = text[b].astype(np.float32)
            for k in range(NK):
                if bands[(r, k)] is None:
                    continue
                lo = bands[(r, k)][0]
                sl = slice(k * 128, (k + 1) * 128)
                j = (r * NK + k) * 4
                cp = cprime[b, sl] - lo      # band-local center
                params[cid, :, j + 0] = cp
                params[cid, :, j + 1] = inv_sig[b, sl]
                params[cid, :, j + 2] = -cp * inv_sig[b, sl]
                params[cid, :, j + 3] = b2[b, sl]
    return params, textf, bands, ncov, L


def _build_program(bands, ncov, reps=1, loops=0):
    import concourse.bacc as bacc
    import concourse.mybir as mybir
    import concourse.tile as tile

    dt = mybir.dt

    nc = bacc.Bacc("TRN2", target_bir_lowering=False, debug=False,
                   num_devices=N_CORES)
    d_params = nc.dram_tensor("params", [128, NR * NK * 4], dt.float32,
                              kind="ExternalInput").ap()
    d_idcol = nc.dram_tensor("idcol", [128, 1], dt.float32,
                             kind="ExternalInput").ap()
    d_textf = nc.dram_tensor("textf", [NR, S], dt.float32,
                             kind="ExternalInput").ap()
    d_embaug = nc.dram_tensor("embaug", [IDIM, D + 1], dt.bfloat16,
                              kind="ExternalInput").ap()
    if loops:
        d_out = nc.dram_tensor("out", [128, 16], dt.bfloat16,
                               kind="ExternalOutput").ap()
    else:
        d_out = nc.dram_tensor("out", [NR, T, D], dt.bfloat16,
                               kind="ExternalOutput").ap()

    wmax = max((hi - lo) for v in bands.values() if v for lo, hi in [v])

    with tile.TileContext(nc) as tc:
        with tc.tile_pool(name="const", bufs=1) as cpool:
            iota_t = cpool.tile([128, wmax], dt.float32, tag="iota")
            nc.gpsimd.iota(iota_t[:, :], pattern=[[1, wmax]], base=0,
                           channel_multiplier=0,
                           allow_small_or_imprecise_dtypes=True)
            params_t = cpool.tile([128, NR * NK * 4], dt.float32, tag="par")
            nc.sync.dma_start(out=params_t[:, :], in_=d_params[:, :])
            idcol_t = cpool.tile([128, 1], dt.float32, tag="idc")
            nc.sync.dma_start(out=idcol_t[:, :], in_=d_idcol[:, :])
            embaug_t = cpool.tile([128, 2, D + 1], dt.bfloat16, tag="emb")
            nc.sync.dma_start(
                out=embaug_t[:, :, :],
                in_=d_embaug.rearrange("(c p) n -> p c n", p=128))
            ones1 = cpool.tile([1, 128], dt.float32, tag="ones")
            nc.gpsimd.memset(ones1[:, :], 1.0)
            onesb = cpool.tile([128, 1], dt.bfloat16, tag="onesb")
            nc.gpsimd.memset(onesb[:, :], 1.0)

            if loops:
                # timing variant: body repeated `loops` times on device,
                # writing internal DRAM scratch; tiny external output.
                d_s = nc.dram_tensor("scratch_out", [NR, T, D],
                                     dt.bfloat16).ap()
                with tc.For_i(0, loops, 1):
                    _emit_body(nc, tc, bands, ncov, d_textf, d_s, iota_t,
                               params_t, idcol_t, embaug_t, ones1, onesb, 0)
                fin = cpool.tile([128, 16], dt.bfloat16, tag="fin")
                nc.vector.memset(fin[:, :], 1.0)
                nc.sync.dma_start(out=d_out[:, :], in_=fin[:, :])
            else:
                for rep in range(reps):
                    d_o = d_out if rep == 0 else nc.dram_tensor(
                        f"scratch_out{rep}", [NR, T, D], dt.bfloat16).ap()
                    _emit_body(nc, tc, bands, ncov, d_textf, d_o, iota_t,
                               params_t, idcol_t, embaug_t, ones1, onesb,
                               rep)
    nc.compile()
    return nc


def _emit_body(nc, tc, bands, ncov, d_textf, d_out, iota_t, params_t,
               idcol_t, embaug_t, ones1, onesb, rep):
    from contextlib import ExitStack
    import concourse.mybir as mybir

    dt = mybir.dt
    Act = mybir.ActivationFunctionType
    Alu = mybir.AluOpType
    cfg = CONFIG
    abl = cfg.get("ablate", "full")
    nci = [0]
    evi = [0]
    ohi = [0]

    def cyc(pat, i):
        c = pat[i[0] % len(pat)]
        i[0] += 1
        return c

    with ExitStack() as ctx:
        embg_pool = ctx.enter_context(
            tc.tile_pool(name=f"embg{rep}", bufs=NR * NK))
        sbuf = ctx.enter_context(
            tc.tile_pool(name=f"work{rep}", bufs=cfg["wbufs"]))
        wpool = ctx.enter_context(tc.tile_pool(name=f"wt{rep}", bufs=2 * NK))
        psum = ctx.enter_context(
            tc.tile_pool(name=f"ps{rep}", bufs=cfg["psbufs"], space="PSUM"))
        psum_sm = ctx.enter_context(
            tc.tile_pool(name=f"pss{rep}", bufs=2, space="PSUM"))

        embg = {}
        wt = {}
        for r in range(NR):
            if abl == "none":
                break
            # ---- gather: one-hot matmul -> emb_g[r][k] ----
            if abl not in ("wonly",):
                trow = sbuf.tile([1, S], dt.float32, tag="trow")
                nc.sync.dma_start(out=trow[:, :], in_=d_textf[r:r + 1, :])
                if cfg["bcast"] == "pe":
                    tbp = psum.tile([128, S], dt.float32, tag="big")
                    nc.tensor.matmul(tbp[:, :], ones1[:, :], trow[:, :],
                                     start=True, stop=True)
                    tb = tbp
                else:
                    tb = sbuf.tile([128, S], dt.float32, tag="tb")
                    nc.gpsimd.partition_broadcast(tb[:, :], trow[:, :])
                oh = []
                for cch in range(IDIM // 128):
                    o = sbuf.tile([128, S], dt.bfloat16, tag="oh")
                    e = cyc(cfg["oh_pat"], ohi)
                    # GPSIMD cannot read PSUM (PE-broadcast tb is PSUM)
                    eng = (nc.vector
                           if (e == "V" or cfg["bcast"] == "pe")
                           else nc.gpsimd)
                    eng.tensor_scalar(
                        o[:, :], tb[:, :], idcol_t[:, 0:1], float(128 * cch),
                        Alu.subtract, Alu.is_equal)
                    oh.append(o)
                for k in range(NK):
                    eg = psum_sm.tile([128, D + 1], dt.float32, tag="egden")
                    for cch in range(IDIM // 128):
                        nc.tensor.matmul(
                            eg[:, :], oh[cch][:, k * 128:(k + 1) * 128],
                            embaug_t[:, cch, :],
                            start=(cch == 0), stop=(cch == 1))
                    g = embg_pool.tile([128, D + 1], dt.bfloat16, tag="embg")
                    if cyc(cfg["ev_pat"], evi) == "V":
                        nc.vector.tensor_copy(g[:, :], eg[:, :])
                    else:
                        nc.scalar.copy(g[:, :], eg[:, :])
                    embg[(r, k)] = g

            # ---- w tiles for this row ----
            for k in range(NK):
                if bands[(r, k)] is None:
                    continue
                lo, hi = bands[(r, k)]
                W = hi - lo
                j = (r * NK + k) * 4
                mode = cfg["w_mode"][r * NK + k]
                sq = sbuf.tile([128, W], dt.bfloat16, tag="sq")
                if mode == "A":
                    nc.scalar.activation(sq[:, :], iota_t[:, 0:W],
                                         Act.Square,
                                         bias=params_t[:, j + 2:j + 3],
                                         scale=params_t[:, j + 1:j + 2])
                else:
                    z = sbuf.tile([128, W], dt.bfloat16, tag="z")
                    zeng = nc.gpsimd if mode in "PG" else nc.vector
                    seng = nc.vector if mode in "VG" else nc.gpsimd
                    zeng.tensor_scalar(z[:, :], iota_t[:, 0:W],
                                       params_t[:, j + 0:j + 1],
                                       params_t[:, j + 1:j + 2],
                                       Alu.subtract, Alu.mult)
                    seng.tensor_tensor(sq[:, :], z[:, :], z[:, :], Alu.mult)
                w = wpool.tile([128, W], dt.bfloat16, tag="w")
                nc.scalar.activation(w[:, :], sq[:, :], Act.Exp,
                                     bias=params_t[:, j + 3:j + 4],
                                     scale=-0.5)
                wt[(r, k)] = (w, lo, hi)

            if abl in ("wonly", "nomm"):
                continue

            # ---- main matmuls + normalize + out DMA ----
            GRP = cfg["grp"]
            for g0 in range(0, ncov[r], GRP):
                g = min(GRP, ncov[r] - g0)
                big = psum.tile([128, GRP * 512], dt.float32, tag="big")
                for q in range(g):
                    tt = g0 + q
                    ks = [k for k in range(NK)
                          if bands[(r, k)]
                          and bands[(r, k)][0] < (tt + 1) * 128
                          and bands[(r, k)][1] > tt * 128]
                    dst = big[:, q * 512:q * 512 + D + 1]
                    for ji, k in enumerate(ks):
                        w, lo, hi = wt[(r, k)]
                        off = tt * 128 - lo
                        nc.tensor.matmul(dst, w[:, off:off + 128],
                                         embg[(r, k)][:, :],
                                         start=(ji == 0),
                                         stop=(ji == len(ks) - 1))
                if abl == "nonorm":
                    continue
                rec = sbuf.tile([128, 8], dt.float32, tag="rec")
                nc.vector.reciprocal(
                    rec[:, 0:g], big[:, D:D + (g - 1) * 512 + 1:512])
                stage = sbuf.tile([128, GRP * D], dt.bfloat16, tag="stage")
                if cyc(cfg["nc_pat"], nci) == "V":
                    nc.vector.tensor_tensor(
                        stage[:, 0:g * D].rearrange("p (g d) -> p g d", g=g),
                        big[:, 0:g * 512].rearrange(
                            "p (g x) -> p g x", g=g)[:, :, 0:D],
                        rec[:, 0:g, None].broadcast_to([128, g, D]),
                        Alu.mult)
                else:
                    for q in range(g):
                        nc.scalar.mul(stage[:, q * D:(q + 1) * D],
                                      big[:, q * 512:q * 512 + D],
                                      rec[:, q:q + 1])
                nc.sync.dma_start(
                    out=d_out[r, g0 * 128:(g0 + g) * 128, :].rearrange(
                        "(q p) d -> p q d", p=128),
                    in_=stage[:, 0:g * D])


def _make_runner(nc):
    import jax
    import concourse.mybir as mybir
    from concourse import bass2jax
    from jax.experimental.shard_map import shard_map
    from jax.sharding import Mesh, PartitionSpec

    bass2jax.install_neuronx_cc_hook()

    partition_name = (nc.partition_id_tensor.name
                      if nc.partition_id_tensor else None)
    in_names, out_names, out_avals, out_shapes = [], [], [], []
    for alloc in nc.m.functions[0].allocations:
        if not isinstance(alloc, mybir.MemoryLocationSet):
            continue
        name = alloc.memorylocations[0].name
        if alloc.kind == "ExternalInput":
            if name != partition_name:
                in_names.append(name)
        elif alloc.kind == "ExternalOutput":
            out_names.append(name)
            shape = tuple(alloc.tensor_shape)
            dtype = mybir.dt.np(alloc.dtype)
            out_avals.append(jax.core.ShapedArray(shape, dtype))
            out_shapes.append((shape, dtype))
    n_params = len(in_names)
    all_in_names = list(in_names) + list(out_names)
    if partition_name is not None:
        all_in_names.append(partition_name)
    all_in_names = tuple(all_in_names)
    donate = tuple(range(n_params, n_params + len(out_names)))

    def _body(*args):
        operands = list(args)
        if partition_name is not None:
            operands.append(bass2jax.partition_id_tensor())
        outs = bass2jax._bass_exec_p.bind(
            *operands,
            out_avals=tuple(out_avals),
            in_names=all_in_names,
            out_names=tuple(out_names),
            lowering_input_output_aliases=(),
            sim_require_finite=True,
            sim_require_nnan=True,
            nc=nc,
        )
        return tuple(outs)

    devices = jax.devices()[:N_CORES]
    mesh = Mesh(np.asarray(devices), ("core",))
    specs = (PartitionSpec("core"),) * (n_params + len(out_names))
    out_specs = (PartitionSpec("core"),) * len(out_names)
    fn = jax.jit(
        shard_map(_body, mesh=mesh, in_specs=specs, out_specs=out_specs,
                  check_rep=False),
        donate_argnums=donate, keep_unused=True)

    state = {"prev_outs": None}

    def run(in_maps):
        concat_in = [
            np.concatenate([np.asarray(in_maps[c][n]) for c in range(N_CORES)],
                           axis=0)
            for n in in_names
        ]
        if state["prev_outs"] is not None:
            out_args = state["prev_outs"]
        else:
            out_args = [
                np.zeros((N_CORES * sh[0], *sh[1:]), dtp)
                for (sh, dtp) in out_shapes
            ]
        out_arrs = fn(*concat_in, *out_args)
        result = {
            name: np.asarray(out_arrs[i]).reshape(N_CORES, *out_shapes[i][0])
            for i, name in enumerate(out_names)
        }
        # donate this call's device-resident outputs back as next call's
        # output buffers (avoids re-uploading zeros through the tunnel)
        state["prev_outs"] = list(out_arrs)
        return result

    return run


def get_runner(key, bands, ncov, reps=1, loops=0):
    k = (key, reps, loops)
    if k not in _runners:
        nc = _build_program(bands, ncov, reps=reps, loops=loops)
        _runners[k] = _make_runner(nc)
    return _runners[k]


def kernel(text, durs, embed, total_time):
    import ml_dtypes

    text = np.asarray(text)
    durs = np.asarray(durs)
    embed = np.asarray(embed, dtype=np.float32)
    tt_ = int(np.asarray(total_time))
    assert tt_ == T and text.shape == (B, S) and embed.shape == (IDIM, D)

    params, textf, bands, ncov, L = _host_prep(text, durs)
    key = tuple(sorted((rk, v) for rk, v in bands.items())) + tuple(ncov)
    run = get_runner(key, bands, ncov)

    embaug = np.ones((IDIM, D + 1), dtype=ml_dtypes.bfloat16)
    embaug[:, :D] = embed.astype(ml_dtypes.bfloat16)
    embaug[PAD, :D] = 0
    idcol = np.arange(128, dtype=np.float32).reshape(128, 1)

    in_maps = [
        {"params": params[cid], "idcol": idcol, "textf": textf[cid],
         "embaug": embaug}
        for cid in range(N_CORES)
    ]
    res = run(in_maps)
    out = res["out"].astype(np.float32).reshape(B, T, D)
    for b in range(B):
        out[b, L[b]:, :] = 0.0
    return out
